# revision 1
# baseline (speedup 1.0000x reference)
"""Trainium2 Bass kernel: 4-layer dense transformer (B=2, T=2048, D=512, H=8, V=32000).

Sharding (DP2 x TP4 over 8 cores): core c handles batch b=c//4, TP rank r=c%4.
Per core: 2 attention heads (2r, 2r+1) over its whole batch, and an 8000-row
vocab shard of the final projection. Wo/LN/FFN run on all 2048 tokens of the
core's batch (replicated within the 4-core group) - this keeps the SPMD
program free of rank-dependent addressing; the only collective is one
AllGather of per-head attention outputs per layer within [[0-3],[4-7]].

Activations are kept transposed [D-partition, token-free]. Host-side prep in
kernel(): embedding gather + positional add, weight transposes, bf16 casts,
per-core slicing. Device work is bf16 matmuls with fp32 accumulation.
"""
import sys
sys.path.insert(0, "/opt/trn_rl_repo")
import numpy as np
import ml_dtypes

import concourse.bass as bass
import concourse.mybir as mybir
import concourse.tile as tile
from concourse import bacc
from concourse.bass_utils import run_bass_kernel_spmd
from concourse.masks import make_identity

F32 = mybir.dt.float32
BF16 = mybir.dt.bfloat16

N_CORES = 8
GROUPS = [[0, 1, 2, 3], [4, 5, 6, 7]]
P = 128
D = 512            # d_model
T = 2048           # tokens per batch (= per core)
H_PER = 2          # heads per core
DK = 64
L = 4              # layers
FF = 2048          # d_ff
VSH = 8000         # vocab shard per core
DC = D // P        # 4 D-chunks
TC = T // P        # 16 token chunks
TW = T // 512      # 4 token windows of 512
FC = FF // P       # 16 ff chunks
EPS = 1e-5
NEG = -1e30


def build_nc_full():
    nc = bacc.Bacc("TRN2", target_bir_lowering=False, debug=False,
                   num_devices=N_CORES)
    h0T = nc.declare_dram_parameter("h0T", [D, T], F32, isOutput=False)
    wqkvT = nc.declare_dram_parameter("wqkvT", [L, D, 3 * P], BF16, isOutput=False)
    woT = nc.declare_dram_parameter("woT", [L, D, D], BF16, isOutput=False)
    w1T = nc.declare_dram_parameter("w1T", [L, D, FF], BF16, isOutput=False)
    w2T = nc.declare_dram_parameter("w2T", [L, FF, D], BF16, isOutput=False)
    ln1g = nc.declare_dram_parameter("ln1g", [L, P, DC], F32, isOutput=False)
    ln1b = nc.declare_dram_parameter("ln1b", [L, P, DC], F32, isOutput=False)
    ln2g = nc.declare_dram_parameter("ln2g", [L, P, DC], F32, isOutput=False)
    ln2b = nc.declare_dram_parameter("ln2b", [L, P, DC], F32, isOutput=False)
    b1v = nc.declare_dram_parameter("b1v", [L, P, FC], F32, isOutput=False)
    b2v = nc.declare_dram_parameter("b2v", [L, P, DC], F32, isOutput=False)
    lnfg = nc.declare_dram_parameter("lnfg", [P, DC], F32, isOutput=False)
    lnfb = nc.declare_dram_parameter("lnfb", [P, DC], F32, isOutput=False)
    outwT = nc.declare_dram_parameter("outwT", [D, VSH], BF16, isOutput=False)
    logits = nc.declare_dram_parameter("logits", [T, VSH], F32, isOutput=True)

    from contextlib import ExitStack
    with tile.TileContext(nc) as tc:
        with ExitStack() as ctx:
            ep = ctx.enter_context
            const = ep(tc.tile_pool(name="const", bufs=1))
            hpool = ep(tc.tile_pool(name="hpool", bufs=1))
            awin = ep(tc.tile_pool(name="awin", bufs=1))
            qkvp = ep(tc.tile_pool(name="qkvp", bufs=1))
            wgt = ep(tc.tile_pool(name="wgt", bufs=1))
            wgt2 = ep(tc.tile_pool(name="wgt2", bufs=2))
            vecs = ep(tc.tile_pool(name="vecs", bufs=3))
            strow = ep(tc.tile_pool(name="strow", bufs=1))
            lnbig = ep(tc.tile_pool(name="lnbig", bufs=4))
            lnwin = ep(tc.tile_pool(name="lnwin", bufs=2))
            smallp = ep(tc.tile_pool(name="small", bufs=2))
            ptp = ep(tc.tile_pool(name="pt", bufs=17))
            vxp = ep(tc.tile_pool(name="vx", bufs=34))
            unp = ep(tc.tile_pool(name="un", bufs=3))
            utp = ep(tc.tile_pool(name="ut", bufs=1))
            utwp = ep(tc.tile_pool(name="utw", bufs=4))
            z1p = ep(tc.tile_pool(name="z1w", bufs=17))
            owp = ep(tc.tile_pool(name="ow", bufs=4))
            psm = ep(tc.tile_pool(name="ps", bufs=3, space="PSUM"))
            pstr = ep(tc.tile_pool(name="pstr", bufs=2, space="PSUM"))
            psov = ep(tc.tile_pool(name="psov", bufs=2, space="PSUM"))
            pstat = ep(tc.tile_pool(name="pst", bufs=1, space="PSUM"))
            dram = ep(tc.tile_pool(name="dram", bufs=2, space="DRAM"))
            # ---- constants ----
            ident = const.tile([P, P], BF16, tag="ident")
            make_identity(nc, ident)
            cmaskT = const.tile([P, P], F32, tag="cmaskT")
            nc.gpsimd.memset(cmaskT[:], 0.0)
            nc.gpsimd.affine_select(
                out=cmaskT[:], in_=cmaskT[:],
                compare_op=mybir.AluOpType.is_ge, fill=NEG,
                base=0, pattern=[[1, P]], channel_multiplier=-1,
            )
            mean_lhs = const.tile([P, 1], F32, tag="mean_lhs")
            nc.gpsimd.memset(mean_lhs[:], 1.0 / D)
            ones_row = const.tile([1, P], F32, tag="ones_row")
            nc.gpsimd.memset(ones_row[:], 1.0)
            eps_t = const.tile([P, 1], F32, tag="eps_t")
            nc.gpsimd.memset(eps_t[:], EPS)

            hT = [hpool.tile([P, T], F32, tag=f"hT{c}", name=f"hT{c}")
                  for c in range(DC)]
            for c in range(DC):
                nc.sync.dma_start(hT[c][:], h0T[c * P:(c + 1) * P, :])

            def load_vec(src, l, w, tag):
                t = vecs.tile([P, w], F32, tag=tag)
                nc.gpsimd.dma_start(t[:], src[l] if l is not None else src[:, :])
                return t

            def ln_window(g_t, b_t, w, out4, osl=None, pfx="ln"):
                """LayerNorm over D for token window w; writes 4 out tiles."""
                sl = slice(w * 512, (w + 1) * 512)
                s01 = lnbig.tile([P, 512], F32, tag="lnbig", name=f"{pfx}s01_{w}")
                s23 = lnbig.tile([P, 512], F32, tag="lnbig", name=f"{pfx}s23_{w}")
                nc.vector.tensor_add(s01[:], hT[0][:, sl], hT[1][:, sl])
                nc.vector.tensor_add(s23[:], hT[2][:, sl], hT[3][:, sl])
                nc.vector.tensor_add(s01[:], s01[:], s23[:])
                q0 = lnbig.tile([P, 512], F32, tag="lnbig", name=f"{pfx}q0_{w}")
                q1 = lnbig.tile([P, 512], F32, tag="lnbig", name=f"{pfx}q1_{w}")
                nc.vector.tensor_tensor(out=q0[:], in0=hT[0][:, sl],
                                        in1=hT[0][:, sl], op=mybir.AluOpType.mult)
                for c in range(1, DC):
                    nc.vector.tensor_tensor(out=q1[:], in0=hT[c][:, sl],
                                            in1=hT[c][:, sl],
                                            op=mybir.AluOpType.mult)
                    nc.vector.tensor_add(q0[:], q0[:], q1[:])
                mp = pstat.tile([1, 512], F32, space="PSUM", tag="st")
                nc.tensor.matmul(mp[:], mean_lhs[:], s01[:], start=True, stop=True)
                mu_row = strow.tile([1, 512], F32, tag="mu_row")
                nc.scalar.copy(mu_row[:], mp[:])
                mp2 = pstat.tile([1, 512], F32, space="PSUM", tag="st")
                nc.tensor.matmul(mp2[:], mean_lhs[:], q0[:], start=True, stop=True)
                ms_row = strow.tile([1, 512], F32, tag="ms_row")
                nc.scalar.copy(ms_row[:], mp2[:])
                bp = psm.tile([P, 512], F32, space="PSUM", tag="mm")
                nc.tensor.matmul(bp[:], ones_row[:], mu_row[:], start=True, stop=True)
                mu_bc = lnwin.tile([P, 512], F32, tag="mu_bc")
                nc.vector.tensor_copy(mu_bc[:], bp[:])
                bp2 = psm.tile([P, 512], F32, space="PSUM", tag="mm")
                nc.tensor.matmul(bp2[:], ones_row[:], ms_row[:], start=True, stop=True)
                rstd = lnwin.tile([P, 512], F32, tag="rstd")
                nc.vector.tensor_tensor(out=rstd[:], in0=mu_bc[:], in1=mu_bc[:],
                                        op=mybir.AluOpType.mult)
                nc.vector.tensor_tensor(out=rstd[:], in0=bp2[:], in1=rstd[:],
                                        op=mybir.AluOpType.subtract)
                nc.scalar.activation(rstd[:], rstd[:],
                                     mybir.ActivationFunctionType.Sqrt,
                                     bias=eps_t[:])
                nc.vector.reciprocal(rstd[:], rstd[:])
                for c in range(DC):
                    tt = smallp.tile([P, 512], F32, tag="ln_app")
                    nc.vector.tensor_tensor(out=tt[:], in0=hT[c][:, sl],
                                            in1=mu_bc[:],
                                            op=mybir.AluOpType.subtract)
                    nc.vector.tensor_tensor(out=tt[:], in0=tt[:], in1=rstd[:],
                                            op=mybir.AluOpType.mult)
                    dst = out4[c][:, osl] if osl is not None else out4[c][:]
                    nc.vector.tensor_scalar(
                        out=dst, in0=tt[:],
                        scalar1=g_t[:, c:c + 1], scalar2=b_t[:, c:c + 1],
                        op0=mybir.AluOpType.mult, op1=mybir.AluOpType.add)

            for l in range(L):
                g1 = load_vec(ln1g, l, DC, "g1")
                bb1 = load_vec(ln1b, l, DC, "bb1")
                g2 = load_vec(ln2g, l, DC, "g2")
                bb2 = load_vec(ln2b, l, DC, "bb2")
                fb1 = load_vec(b1v, l, FC, "fb1")
                fb2 = load_vec(b2v, l, DC, "fb2")
                wq_sb = [wgt2.tile([P, 3 * P], BF16, tag=f"wq{k}", name=f"wq{k}_{l}")
                         for k in range(DC)]
                wo_sb = [wgt2.tile([P, D], BF16, tag=f"wo{k}", name=f"wo{k}_{l}")
                         for k in range(DC)]
                w1_sb = [wgt.tile([P, FF], BF16, tag=f"w1{k}", name=f"w1{k}_{l}")
                         for k in range(DC)]
                w2_sb = [wgt.tile([P, D], BF16, tag=f"w2{k}", name=f"w2{k}_{l}")
                         for k in range(FC)]
                for k in range(DC):
                    nc.gpsimd.dma_start(wq_sb[k][:], wqkvT[l, k * P:(k + 1) * P, :])
                    nc.gpsimd.dma_start(wo_sb[k][:], woT[l, k * P:(k + 1) * P, :])
                    nc.gpsimd.dma_start(w1_sb[k][:], w1T[l, k * P:(k + 1) * P, :])
                for k in range(FC):
                    nc.gpsimd.dma_start(w2_sb[k][:], w2T[l, k * P:(k + 1) * P, :])

                # ---- LN1 + QKV, windowed ----
                qkv_sb = [qkvp.tile([P, T], BF16, tag=f"qkv{m}", name=f"qkv{m}_{l}")
                          for m in range(3)]
                for w in range(TW):
                    aw = [awin.tile([P, 512], BF16, tag=f"aw{c}",
                                    name=f"aw{c}_{l}_{w}") for c in range(DC)]
                    ln_window(g1, bb1, w, aw, pfx=f"l1_{l}")
                    for m in range(3):
                        pp = psm.tile([P, 512], F32, space="PSUM", tag="mm")
                        for k in range(DC):
                            nc.tensor.matmul(
                                pp[:], wq_sb[k][:, m * P:(m + 1) * P], aw[k][:],
                                start=(k == 0), stop=(k == DC - 1))
                        nc.scalar.copy(qkv_sb[m][:, w * 512:(w + 1) * 512], pp[:])

                # ---- attention, 2 heads ----
                uT = utp.tile([P, T], BF16, tag="uT")
                for h in range(H_PER):
                    hs = slice(h * DK, (h + 1) * DK)
                    vx = []
                    for kj in range(TC):
                        vt = pstr.tile([P, P], BF16, space="PSUM", tag="tr")
                        nc.tensor.transpose(
                            out=vt[:, :DK],
                            in_=qkv_sb[2][hs, kj * P:(kj + 1) * P],
                            identity=ident[hs, hs])
                        vxt = vxp.tile([P, DK + 1], BF16, tag="vx")
                        nc.vector.tensor_copy(vxt[:, :DK], vt[:, :DK])
                        nc.vector.memset(vxt[:, DK:DK + 1], 1.0)
                        vx.append(vxt)
                    for w in range(TW):
                        qsl = slice(w * 512, (w + 1) * 512)
                        pts = {}
                        for kj in range((w + 1) * 4):
                            sp = psm.tile([P, 512], F32, space="PSUM", tag="mm")
                            nc.tensor.matmul(
                                sp[:], qkv_sb[1][hs, kj * P:(kj + 1) * P],
                                qkv_sb[0][hs, qsl], start=True, stop=True)
                            if kj >= w * 4:
                                off = kj * P - w * 512
                                nc.vector.tensor_add(
                                    sp[:, off:off + P], sp[:, off:off + P],
                                    cmaskT[:])
                            pt = ptp.tile([P, 512], BF16, tag="pt")
                            nc.scalar.activation(
                                pt[:], sp[:], mybir.ActivationFunctionType.Exp,
                                bias=0.0, scale=0.125)
                            pts[kj] = pt
                        for qc in range(4):
                            qi = w * 4 + qc
                            op = psov.tile([P, DK + 1], F32, space="PSUM", tag="ov")
                            for kj in range(qi + 1):
                                nc.tensor.matmul(
                                    op[:], pts[kj][:, qc * P:(qc + 1) * P],
                                    vx[kj][:], start=(kj == 0), stop=(kj == qi))
                            rl = smallp.tile([P, 1], F32, tag="rl")
                            nc.vector.reciprocal(rl[:], op[:, DK:DK + 1])
                            un = unp.tile([P, P], BF16, tag="un")
                            nc.vector.tensor_scalar(
                                out=un[:, hs], in0=op[:, :DK],
                                scalar1=rl[:, :1], scalar2=None,
                                op0=mybir.AluOpType.mult)
                            tp = pstr.tile([P, P], BF16, space="PSUM", tag="tr")
                            nc.tensor.transpose(out=tp[:], in_=un[:],
                                                identity=ident[:])
                            nc.vector.tensor_copy(uT[hs, qi * P:(qi + 1) * P], tp[hs, :])

                # ---- AG#1 ----
                ag1_in = dram.tile([P, T], BF16, tag="ag1_in")
                ag1_out = dram.tile([4 * P, T], BF16, tag="ag1_out")
                nc.sync.dma_start(ag1_in[:], uT[:])
                nc.gpsimd.collective_compute(
                    "AllGather", mybir.AluOpType.bypass,
                    replica_groups=GROUPS,
                    ins=[ag1_in[:].opt()], outs=[ag1_out[:].opt()])

                # ---- Wo + residual; then LN2 + fused FFN; per window ----
                for n in range(TW):
                    sl = slice(n * 512, (n + 1) * 512)
                    utw = [utwp.tile([P, 512], BF16, tag="utw",
                                     name=f"utw{l}_{n}_{k2}") for k2 in range(DC)]
                    for k in range(DC):
                        nc.sync.dma_start(utw[k][:], ag1_out[k * P:(k + 1) * P, sl])
                    for m in range(DC):
                        pp = psm.tile([P, 512], F32, space="PSUM", tag="mm")
                        for k in range(DC):
                            nc.tensor.matmul(
                                pp[:], wo_sb[k][:, m * P:(m + 1) * P], utw[k][:],
                                start=(k == 0), stop=(k == DC - 1))
                        nc.vector.tensor_add(hT[m][:, sl], hT[m][:, sl], pp[:])
                for n in range(TW):
                    sl = slice(n * 512, (n + 1) * 512)
                    a2 = [awin.tile([P, 512], BF16, tag=f"a2w{c}",
                                    name=f"a2w{c}_{l}_{n}") for c in range(DC)]
                    ln_window(g2, bb2, n, a2, pfx=f"l2_{l}")
                    z1g = [z1p.tile([P, 512], BF16, tag="z1w",
                                     name=f"z1g{l}_{n}_{m}") for m in range(FC)]
                    for m in range(FC):
                        pp = psm.tile([P, 512], F32, space="PSUM", tag="mm")
                        for k in range(DC):
                            nc.tensor.matmul(
                                pp[:], w1_sb[k][:, m * P:(m + 1) * P], a2[k][:],
                                start=(k == 0), stop=(k == DC - 1))
                        nc.scalar.activation(
                            z1g[m][:], pp[:], mybir.ActivationFunctionType.Gelu,
                            bias=fb1[:, m:m + 1])
                    for md in range(DC):
                        pp = psm.tile([P, 512], F32, space="PSUM", tag="mm")
                        for k in range(FC):
                            nc.tensor.matmul(
                                pp[:], w2_sb[k][:, md * P:(md + 1) * P], z1g[k][:],
                                start=(k == 0), stop=(k == FC - 1))
                        tt = smallp.tile([P, 512], F32, tag="ffn_out")
                        nc.vector.tensor_scalar(
                            out=tt[:], in0=pp[:], scalar1=fb2[:, md:md + 1],
                            scalar2=None, op0=mybir.AluOpType.add)
                        nc.vector.tensor_add(hT[md][:, sl], hT[md][:, sl], tt[:])

            # ---- final LN + vocab-shard projection ----
            gf = load_vec(lnfg, None, DC, "gf")
            bf_t = load_vec(lnfb, None, DC, "bf")
            afT = [qkvp.tile([P, T], BF16, tag=f"qkv{c}", name=f"afT{c}")
                   for c in range(3)]
            afT.append(utp.tile([P, T], BF16, tag="uT", name="afT3"))
            for w in range(TW):
                ln_window(gf, bf_t, w, afT, osl=slice(w * 512, (w + 1) * 512),
                          pfx="lnf")
            NV = 500
            for vc in range(VSH // NV):
                ow_sb = [owp.tile([P, NV], BF16, tag="ow", name=f"ow{vc}_{k2}")
                         for k2 in range(DC)]
                for k in range(DC):
                    nc.gpsimd.dma_start(
                        ow_sb[k][:],
                        outwT[k * P:(k + 1) * P, vc * NV:(vc + 1) * NV])
                for tcx in range(TC):
                    pp = psm.tile([P, 512], F32, space="PSUM", tag="mm")
                    for k in range(DC):
                        nc.tensor.matmul(
                            pp[:, :NV], afT[k][:, tcx * P:(tcx + 1) * P],
                            ow_sb[k][:], start=(k == 0), stop=(k == DC - 1))
                    lo = smallp.tile([P, NV], F32, tag="lo", name=f"lo{vc}_{tcx}")
                    if tcx % 2 == 0:
                        nc.scalar.copy(lo[:], pp[:, :NV])
                    else:
                        nc.vector.tensor_copy(lo[:], pp[:, :NV])
                    nc.sync.dma_start(
                        logits[tcx * P:(tcx + 1) * P, vc * NV:(vc + 1) * NV],
                        lo[:])
    nc.compile()
    return nc



_NC_CACHE = None


def _get_nc():
    global _NC_CACHE
    if _NC_CACHE is None:
        _NC_CACHE = build_nc_full()
    return _NC_CACHE


def _vec_tile(v, chunks):
    # [chunks*128] -> [128, chunks] with [p, c] = v[c*128+p]
    return np.ascontiguousarray(np.asarray(v, np.float32).reshape(chunks, P).T)


def prepare_in_maps(inputs):
    return _prep(**inputs)


def _prep(x, embed_w, pos_w, ln1_g, ln1_b, Wqkv, Wo, ln2_g, ln2_b,
          W1, b1, W2, b2, lnf_g, lnf_b, out_w):
    x = np.asarray(x)
    embed_w = np.asarray(embed_w, np.float32)
    pos_w = np.asarray(pos_w, np.float32)
    Wqkv = np.asarray(Wqkv, np.float32)
    bf = ml_dtypes.bfloat16
    woT = np.ascontiguousarray(np.asarray(Wo).transpose(0, 2, 1)).astype(bf)
    w1T = np.ascontiguousarray(np.asarray(W1).transpose(0, 2, 1)).astype(bf)
    w2T = np.ascontiguousarray(np.asarray(W2).transpose(0, 2, 1)).astype(bf)
    ln_tiles = {
        "ln1g": np.stack([_vec_tile(np.asarray(ln1_g)[l], DC) for l in range(L)]),
        "ln1b": np.stack([_vec_tile(np.asarray(ln1_b)[l], DC) for l in range(L)]),
        "ln2g": np.stack([_vec_tile(np.asarray(ln2_g)[l], DC) for l in range(L)]),
        "ln2b": np.stack([_vec_tile(np.asarray(ln2_b)[l], DC) for l in range(L)]),
        "b1v": np.stack([_vec_tile(np.asarray(b1)[l], FC) for l in range(L)]),
        "b2v": np.stack([_vec_tile(np.asarray(b2)[l], DC) for l in range(L)]),
        "lnfg": _vec_tile(lnf_g, DC),
        "lnfb": _vec_tile(lnf_b, DC),
    }
    in_maps = []
    for c in range(N_CORES):
        b, r = c // 4, c % 4
        h0 = embed_w[x[b]] + pos_w[:T]                       # [T, D]
        h0T = np.ascontiguousarray(h0.T).astype(np.float32)  # [D, T]
        heads = [2 * r, 2 * r + 1]
        rows = np.concatenate([
            np.r_[heads[0] * DK:(heads[0] + 1) * DK,
                  heads[1] * DK:(heads[1] + 1) * DK] + w * D
            for w in range(3)])
        wqkvT = np.ascontiguousarray(
            Wqkv[:, rows, :].transpose(0, 2, 1)).astype(bf)  # [L, 512, 384]
        outwT = np.ascontiguousarray(
            np.asarray(out_w)[r * VSH:(r + 1) * VSH].T).astype(bf)
        m = {"h0T": h0T, "wqkvT": wqkvT, "woT": woT, "w1T": w1T, "w2T": w2T,
             "outwT": outwT}
        m.update(ln_tiles)
        in_maps.append(m)
    return in_maps


def kernel(**inputs):
    nc = _get_nc()
    in_maps = prepare_in_maps(inputs)
    res = run_bass_kernel_spmd(nc, in_maps, list(range(N_CORES)))
    out = np.empty((2, T, 4 * VSH), np.float32)
    for c in range(N_CORES):
        b, r = c // 4, c % 4
        out[b, :, r * VSH:(r + 1) * VSH] = res.results[c]["logits"]
    return out

